# revision 37
# baseline (speedup 1.0000x reference)
"""Trainium2 Bass kernel for nn_Block_34067680592489.

Computes, for B=32768 independent signals x[b] (length 256):
  mu,reg = small-CNN(x[b])      (conv5+avgpool4+softplus twice, linear, softplus)
  grad   = TtT x - x_b + reg * DtD x
  x_t    = x - gamma * grad,  gamma = softplus(gamma_p)
  out    = middle root of z^3 -(m+x_t) z^2 + (m x_t - 2 gm) z + gm m,  gm = gamma*mu

Device algorithm (per element, normalized to mass=1, s = (1+xt)/3):
  sqe = (s - 1/2)^2                          (ACT Square, table-free)
  c13 = 2/3*gph + 1/4,  gph = gamma*mu - 1/4 (per-row params from the CNN)
  hm  = sqe + c13   (= -p/3 > 1/12 always; never materialized)
  D'  = hm^3 - sqe*(sqe+gph)^2               (fused DVE op; = dm4/4 > 0)
  r   = 2*sqrt(hm) = Sqrt(4*sqe + 4*c13)     (ACT Sqrt, per-partition bias)
  irs = Rsqrt(D')                            (ACT)
  w   = (s-1/2)*(sqe+gph)*irs  (= -u)        (fused DVE op)
  at  = Arctan(w)                            (ACT)
  root= s - r*sin(at/3)                      (DVE odd-poly * r;  Pool adds s)
The middle-root identity cos((phi-2pi)/3) = -sin(arcsin(w')/3) reduces the
whole trig solve to Arctan+Sin-poly.  Output root is bounded in (0.17, 0.83)
for this data so bf16 I/O is safe (measured 7e-3 max rel vs 2e-2 budget).

Sharding: pure data parallel over batch, 8 cores x 4096 rows.  x arrives
pre-transposed bf16 so the PE contraction dim is on partitions; x_b arrives
pre-scaled f32 and is DMA'd straight into PSUM, where the TtT-part matmuls
accumulate on top of it (start=False), removing the separate add pass.
"""

import numpy as np

B_TOTAL = 32768
N = 256
N_CORES = 8
BC = B_TOTAL // N_CORES      # rows per core
TILES = BC // 128            # 32 batch tiles of 128
CT = 4                       # tiles per elementwise chunk
CHUNKS = TILES // CT         # 8
CF = CT * N                  # chunk free size (1024)

PACK_H2 = True               # pack 4 groups' h2 onto 128 partitions
OUT_BF16 = True              # bf16 output dram tensor
import os
DEBUG_PROBE = os.environ.get('K_PROBE', '')   # '', 's', 'sqe', 'dp', 'w'

_PROG = {}


def _np_f32(a):
    return np.ascontiguousarray(np.asarray(a, dtype=np.float32))


def _conv_pool_mat(w, L):
    """(L/4, L) matrix implementing conv1d(k=5,pad=2) then avgpool4."""
    taps = np.asarray(w, np.float32).reshape(5)
    C = np.zeros((L, L), np.float32)
    for n in range(L):
        for k in range(5):
            m = n + k - 2
            if 0 <= m < L:
                C[n, m] = taps[k]
    P = np.zeros((L // 4, L), np.float32)
    for i in range(L // 4):
        P[i, 4 * i:4 * i + 4] = 0.25
    return (P @ C).astype(np.float32)


_CUSTOM_OPS = {}


def _get_custom_ops():
    """Register this kernel's fused custom-DVE ops (idempotent).

    DMS: D' = (sqe+c13)^3 - sqe*(sqe+gph)^2       (C0=c13, C1=gph)
    WU:  w  = ((s-1/2)*((s-1/2)^2+gph))*irs       (C0=gph, imm2=1/2)
    RGSIN: rg = (at*(s0+at^2*(s1+at^2*imm2)))*r   (odd sin(x/3) poly)
    """
    if _CUSTOM_OPS:
        return _CUSTOM_OPS
    import concourse.dve_ops as dops
    from concourse.dve_spec import (Spec, Src0, Src1, C0, C1, C2, sq,
                                    lower, _has_src1)
    from concourse.dve_uop import DveOpSpec

    def reg(name, spec):
        if name in dops._SUB_OPCODE_FOR_NAME:
            return next(o for o in dops.OPS if o.name == name)
        row = dops._CUSTOM_DVE_ROW_BASE + len(dops.OPS)
        assert row < 0x20
        dops._SUB_OPCODE_FOR_NAME[name] = row
        shas = {}
        for ver in ("v3", "v4"):
            u = lower(spec, ver=ver)
            shas[ver] = DveOpSpec(name=name, opcode=row, uops=u,
                                  rd1_en=_has_src1(spec)).sha(ver)
        op = dops.DveOp(name, spec, subdim=False, uops_sha=shas)
        dops.OPS.append(op)
        dops.CUSTOM_DVE_SPECS[name] = spec
        return op

    import numpy as np_

    _h = Src0 + C0
    _CUSTOM_OPS['DMS'] = reg('ANT_K_DMS', Spec(
        body=(sq(_h) * _h) - Src0 * sq(Src0 + C1),
        reference=lambda in0, in1, s0, s1, imm2:
            (((in0 + s0) ** 2 * (in0 + s0))
             - in0 * (in0 + s1) ** 2).astype(np_.float32),
    ))
    _em = Src0 - C2
    _CUSTOM_OPS['WU'] = reg('ANT_K_WU', Spec(
        body=(_em * (sq(_em) + C0)) * Src1,
        reference=lambda in0, in1, s0, s1, imm2:
            (((in0 - imm2) * ((in0 - imm2) ** 2 + s0)) * in1
             ).astype(np_.float32),
    ))
    _a2 = sq(Src0)
    _CUSTOM_OPS['RGSIN'] = reg('ANT_K_RGSIN', Spec(
        body=(Src0 * (C0 + _a2 * (C1 + _a2 * C2))) * Src1,
        reference=lambda in0, in1, s0, s1, imm2:
            ((in0 * (s0 + in0 * in0 * (s1 + in0 * in0 * imm2))) * in1
             ).astype(np_.float32),
    ))
    return _CUSTOM_OPS


_TABLES_PATCHED = False


def _patch_act_tables():
    """Restrict ACT table-set choice to the sets this kernel uses so the
    chooser binds Exp/Ln -> natural_log_exp_and_others, Sqrt ->
    sqrt_and_others, Rsqrt -> reciprocal_sqrt_and_small, Arctan ->
    trig_and_small (Square is in every set and never forces a load)."""
    global _TABLES_PATCHED
    if _TABLES_PATCHED:
        return
    import concourse.bacc as bacc
    keep = {'natural_log_exp_and_others', 'sqrt_and_others',
            'abs_reciprocal_sqrt_and_small', 'trig_and_small'}
    orig = bacc.get_activation_tables

    def patched(arch):
        t = orig(arch)
        return {k: (v if k in keep else set()) for k, v in t.items()}

    bacc.get_activation_tables = patched
    _TABLES_PATCHED = True


def _build_program():
    import concourse.bacc as bacc
    import concourse.tile as tile
    import concourse.mybir as mybir
    from concourse.tile import add_dep_helper
    _patch_act_tables()

    dt = mybir.dt
    f32 = dt.float32
    bf16 = dt.bfloat16
    Alu = mybir.AluOpType
    AF = mybir.ActivationFunctionType
    odt = bf16 if OUT_BF16 else f32

    COPS = _get_custom_ops()
    nc = bacc.Bacc("TRN2", target_bir_lowering=False, debug=False,
                   num_devices=N_CORES)

    XT = nc.dram_tensor("xt", (256, BC), bf16, kind="ExternalInput")
    XBT = nc.dram_tensor("xbt", (256, BC), bf16, kind="ExternalInput")
    EYE = nc.dram_tensor("eye", (128, 128), bf16, kind="ExternalInput")
    WM = nc.dram_tensor("wm", (256, 512), bf16, kind="ExternalInput")
    M1T = nc.dram_tensor("m1t", (256, 128), bf16, kind="ExternalInput")
    M2BD = nc.dram_tensor("m2bd", (128, 32), bf16, kind="ExternalInput")
    LWBD = nc.dram_tensor("lwbd", (128, 2), bf16, kind="ExternalInput")
    B2V = nc.dram_tensor("b2v", (128, 1), f32, kind="ExternalInput")
    B3V = nc.dram_tensor("b3v", (128, 1), f32, kind="ExternalInput")
    LBM = nc.dram_tensor("lbm", (128, 1), f32, kind="ExternalInput")
    LBR = nc.dram_tensor("lbr", (128, 1), f32, kind="ExternalInput")
    GSC = nc.dram_tensor("gsc", (128, 1), f32, kind="ExternalInput")
    OUT = nc.dram_tensor("out", (BC, 256), odt, kind="ExternalOutput")

    NSG = 2 if PACK_H2 else CHUNKS   # supergroups
    GPS = CHUNKS // NSG              # groups per supergroup

    with tile.TileContext(nc) as tc:
        with (
            tc.tile_pool(name="const", bufs=1) as cpool,
            tc.tile_pool(name="so", bufs=CHUNKS) as sopool,
            tc.tile_pool(name="sq", bufs=CHUNKS + 1) as sqpool,
            tc.tile_pool(name="dp", bufs=CHUNKS) as dppool,
            tc.tile_pool(name="wv", bufs=2) as wvpool,
            tc.tile_pool(name="rr", bufs=CHUNKS) as rrpool,
            tc.tile_pool(name="oo", bufs=2) as oopool,
            tc.tile_pool(name="pm", bufs=4, space="PSUM") as pmpool,
            tc.tile_pool(name="pc1", bufs=2, space="PSUM") as pc1pool,
            tc.tile_pool(name="pc2", bufs=1, space="PSUM") as pc2pool,
            tc.tile_pool(name="pc3", bufs=1, space="PSUM") as pc3pool,
        ):
            # ---- constants into SBUF ----
            wm = cpool.tile([128, 2, 512], bf16)
            m1t = cpool.tile([128, 2, 128], bf16)
            m2bd = cpool.tile([128, 32], bf16)
            lwbd = cpool.tile([128, 2], bf16)
            b2v = cpool.tile([128, 1], f32)
            b3v = cpool.tile([128, 1], f32)
            lbm = cpool.tile([128, 1], f32)
            lbr = cpool.tile([128, 1], f32)
            gsc = cpool.tile([128, 1], f32)
            spE = cpool.tile([128, 2 * TILES], f32)
            sp = cpool.tile([128, 2 * TILES], f32)
            gph = cpool.tile([128, TILES], f32)
            c13p = cpool.tile([128, TILES], f32)
            c13x4 = cpool.tile([128, TILES], f32)
            eye = cpool.tile([128, 128], bf16)
            nc.sync.dma_start(eye[:], EYE[:])
            cm16 = cpool.tile([128, 1], f32)
            nc.vector.memset(cm16[:], -0.5)
            for k in range(2):
                nc.sync.dma_start(m1t[:, k, :], M1T[128 * k:128 * (k + 1), :])
            nc.sync.dma_start(m2bd[:], M2BD[:])
            nc.sync.dma_start(lwbd[:], LWBD[:])
            nc.sync.dma_start(b2v[:], B2V[:])
            nc.sync.dma_start(b3v[:], B3V[:])
            nc.sync.dma_start(lbm[:], LBM[:])
            nc.sync.dma_start(lbr[:], LBR[:])
            nc.sync.dma_start(gsc[:], GSC[:])

            s_chunks = [sopool.tile([128, CF], f32, tag="so", name=f"s{c}")
                        for c in range(CHUNKS)]
            sq_chunks = [None] * CHUNKS
            dp_chunks = [None] * CHUNKS
            w_chunks = [None] * CHUNKS
            r_chunks = [None] * CHUNKS

            with (
                tc.tile_pool(name="xt", bufs=1) as xtpool,
                tc.tile_pool(name="cnn", bufs=2) as cnnpool,
            ):
                # ---- inputs ----
                xt_sb = xtpool.tile([128, 2, BC], bf16)
                xb_sb = xtpool.tile([128, 2, BC], bf16)
                for qq in range(4):
                    qsl = slice(BC // 4 * qq, BC // 4 * (qq + 1))
                    for k in range(2):
                        nc.sync.dma_start(xt_sb[:, k, qsl],
                                          XT[128 * k:128 * (k + 1), qsl])
                    for k in range(2):
                        nc.gpsimd.dma_start(xb_sb[:, k, qsl],
                                            XBT[128 * k:128 * (k + 1), qsl])
                    if qq == 0:
                        for k in range(2):
                            nc.scalar.dma_start(wm[:, k, :],
                                                WM[128 * k:128 * (k + 1), :])

                sp_insts = []
                spEv = spE[:].rearrange("p (t c) -> p c t", c=2)
                spv = sp[:].rearrange("p (t c) -> p c t", c=2)
                p3 = pc3pool.tile([128, 2 * TILES], f32)

                for sg in range(NSG):
                    gs_range = range(GPS * sg, GPS * (sg + 1))
                    # --- CNN front: conv1+softplus per group, conv2 matmul
                    if PACK_H2:
                        p2 = pc2pool.tile([128, 512], f32, tag="p2",
                                          name=f"p2sg{sg}")
                    h1list = []
                    for q, g in enumerate(gs_range):
                        sl = slice(512 * g, 512 * (g + 1))
                        p1 = pc1pool.tile([128, 512], f32, tag="p1",
                                          name=f"p1g{g}")
                        nc.tensor.matmul(p1[:], m1t[:, 0, :],
                                         xt_sb[:, 0, sl],
                                         start=True, stop=False)
                        nc.tensor.matmul(p1[:], m1t[:, 1, :],
                                         xt_sb[:, 1, sl],
                                         start=False, stop=True)
                        eh1 = cnnpool.tile([128, 512], f32, tag="eh1",
                                           name=f"eh1g{g}")
                        nc.scalar.activation(eh1[:], p1[:], AF.Exp,
                                             bias=b2v[:])
                        h1s = cnnpool.tile([128, 512], bf16, tag="h1s",
                                           name=f"h1sg{g}")
                        nc.scalar.activation(h1s[:], eh1[:], AF.Ln, bias=1.0)
                        h1list.append(h1s)
                        if PACK_H2:
                            nc.tensor.matmul(p2[32 * q:32 * (q + 1), :],
                                             m2bd[:], h1s[:],
                                             start=True, stop=True,
                                             tile_position=(0, 32 * q),
                                             skip_group_check=True)
                        else:
                            p2 = pc2pool.tile([32, 512], f32, tag="p2",
                                              name=f"p2g{g}")
                            nc.tensor.matmul(p2[:], m2bd[:], h1s[:],
                                             start=True, stop=True)
                            eh2 = cnnpool.tile([32, 512], f32, tag="eh2",
                                               name=f"eh2g{g}")
                            nc.scalar.activation(eh2[:], p2[:], AF.Exp,
                                                 bias=b3v[0:32, :])
                            h2s = cnnpool.tile([32, 512], bf16, tag="h2s",
                                               name=f"h2sg{g}")
                            nc.scalar.activation(h2s[:], eh2[:], AF.Ln,
                                                 bias=1.0)
                            for i in range(4):
                                t = 4 * g + i
                                nc.tensor.matmul(
                                    p3[:, 2 * t:2 * t + 2],
                                    h2s[:, 128 * i:128 * (i + 1)],
                                    lwbd[0:32, :], start=True, stop=True)
                    if PACK_H2:
                        eh2 = cnnpool.tile([128, 512], f32, tag="eh2",
                                           name=f"eh2sg{sg}")
                        nc.scalar.activation(eh2[:], p2[:], AF.Exp,
                                             bias=b3v[:])
                        h2s = cnnpool.tile([128, 512], bf16, tag="h2s",
                                           name=f"h2ssg{sg}")
                        nc.scalar.activation(h2s[:], eh2[:], AF.Ln, bias=1.0)
                        for q, g in enumerate(gs_range):
                            for i in range(4):
                                t = 4 * g + i
                                nc.tensor.matmul(
                                    p3[:, 2 * t:2 * t + 2],
                                    h2s[32 * q:32 * (q + 1),
                                        128 * i:128 * (i + 1)],
                                    lwbd[32 * q:32 * (q + 1), :],
                                    start=True, stop=True,
                                    tile_position=(32 * q, 0),
                                    skip_group_check=True)

                    # --- supergroup tail: final softplus + per-tile params
                    # (merged across the supergroup: fewer, larger ACT ops)
                    sgt = slice(4 * GPS * sg, 4 * GPS * (sg + 1))
                    sgs = slice(8 * GPS * sg, 8 * GPS * (sg + 1))
                    nc.scalar.activation(spEv[:, 0, sgt],
                                         p3[:, sgs].rearrange(
                                             "p (t c) -> p c t", c=2)[:, 0, :],
                                         AF.Exp, bias=lbm[:])
                    nc.scalar.activation(spEv[:, 1, sgt],
                                         p3[:, sgs].rearrange(
                                             "p (t c) -> p c t", c=2)[:, 1, :],
                                         AF.Exp, bias=lbr[:])
                    sp_i = nc.scalar.activation(sp[:, sgs], spE[:, sgs],
                                                AF.Ln, bias=1.0)
                    sp_insts.append(sp_i)
                    nc.vector.tensor_scalar(gph[:, sgt], spv[:, 0, sgt],
                                            gsc[:], -0.25,
                                            Alu.mult, Alu.add)
                    nc.vector.tensor_scalar(c13p[:, sgt], gph[:, sgt],
                                            2.0 / 3.0, 0.25,
                                            Alu.mult, Alu.add)
                    nc.vector.tensor_scalar(c13x4[:, sgt], gph[:, sgt],
                                            8.0 / 3.0, 1.0,
                                            Alu.mult, Alu.add)

                    # --- per-group: main matmuls, s, sqe, D'
                    for g in gs_range:
                        c = g
                        s_c = s_chunks[c]
                        for t in range(4 * g, 4 * (g + 1)):
                            tsl = slice(128 * t, 128 * (t + 1))
                            pm = pmpool.tile([128, 512], f32, tag="pm",
                                             name=f"pm{t}")
                            # One accumulation group for the whole 2KB bank:
                            # start=True (which resets the full bank) only on
                            # the first matmul, stop=True only on the last.
                            # xb3 enters via identity-moving matmuls; the
                            # TtT/DtD matmuls accumulate on top.
                            for k in range(2):
                                nc.tensor.matmul(
                                    pm[:, 128 * k:128 * (k + 1)],
                                    xb_sb[:, k, tsl], eye[:],
                                    start=(k == 0), stop=False,
                                    skip_group_check=True)
                            for k in range(2):
                                nc.tensor.matmul(
                                    pm[:, 0:256],
                                    xt_sb[:, k, tsl], wm[:, k, 0:256],
                                    start=False, stop=False,
                                    skip_group_check=True)
                                nc.tensor.matmul(
                                    pm[:, 256:512],
                                    xt_sb[:, k, tsl], wm[:, k, 256:512],
                                    start=False, stop=(k == 1),
                                    skip_group_check=True)
                            # DVE may read only ONE operand from PSUM, so
                            # the reg*DtD half goes through ACT (Identity
                            # with per-partition scale; table-free).
                            osl = slice(256 * (t % CT), 256 * (t % CT + 1))
                            td = wvpool.tile([128, 256], f32, tag="td",
                                             name=f"td{t}")
                            nc.vector.tensor_scalar(
                                td[:], pm[:, 256:512],
                                spv[:, 1, t:t + 1], None, Alu.mult)
                            nc.vector.scalar_tensor_tensor(
                                s_c[:, osl], pm[:, 0:256], 1.0 / 3.0,
                                td[:], Alu.add, Alu.add)
                        # sqe on ACT (Square is table-free); s_c is the
                        # centered s' = s - 1/3, so (s-1/2) = s' - 1/6
                        sqe = sqpool.tile([128, CF], f32, tag="sq",
                                          name=f"sq{c}")
                        sq_chunks[c] = sqe
                        nc.scalar.activation(sqe[:], s_c[:], AF.Square,
                                             bias=cm16[:])
                        dp = dppool.tile([128, CF], f32, tag="dp",
                                         name=f"dp{c}")
                        dp_chunks[c] = dp
                        for i in range(CT):
                            t = CT * c + i
                            osl = slice(256 * i, 256 * (i + 1))
                            nc.vector._custom_dve(
                                COPS['DMS'], out=dp[:, osl],
                                in0=sqe[:, osl],
                                s0=c13p[:, t:t + 1], s1=gph[:, t:t + 1])

            # ---- B: sqrt block (r and 2*sqrt(D') share the sqrt table),
            # then fast-reciprocal + w on DVE, then the trig block ----
            first_sqrt = None
            for c in range(CHUNKS):
                r = rrpool.tile([128, CF], f32, tag="rr", name=f"r{c}")
                r_chunks[c] = r
                for i in range(CT):
                    t = CT * c + i
                    osl = slice(256 * i, 256 * (i + 1))
                    sq_i = nc.scalar.activation(r[:, osl],
                                                sq_chunks[c][:, osl],
                                                AF.Sqrt,
                                                bias=c13x4[:, t:t + 1],
                                                scale=4.0)
                    if first_sqrt is None:
                        first_sqrt = sq_i
                        for spi in sp_insts:
                            add_dep_helper(sq_i.ins, spi.ins, sync=False,
                                           reason="sqrt block after NLE")
                    else:
                        add_dep_helper(sq_i.ins, last_sqrt_blk.ins,
                                       sync=False, reason="chain sqrt block")
                    last_sqrt_blk = sq_i
            last_sd2 = None
            last_rsq = None
            for c in range(CHUNKS):
                dp = dp_chunks[c]
                irs_i = nc.scalar.activation(dp[:], dp[:],
                                             AF.Abs_reciprocal_sqrt)
                add_dep_helper(irs_i.ins, last_sqrt_blk.ins, sync=False,
                               reason="absrsqrt block after sqrt block")
                last_sd2 = irs_i
                last_rsq = irs_i
                w = wvpool.tile([128, CF], f32, tag="wv", name=f"w{c}")
                w_chunks[c] = w
                for i in range(CT):
                    t = CT * c + i
                    osl = slice(256 * i, 256 * (i + 1))
                    nc.vector._custom_dve(
                        COPS['WU'], out=w[:, osl], in0=s_chunks[c][:, osl],
                        in1=dp[:, osl], s0=gph[:, t:t + 1], imm2=0.5)
            first_at = None
            for c in range(CHUNKS):
                w = w_chunks[c]
                at_i = nc.scalar.activation(w[:], w[:], AF.Arctan)
                add_dep_helper(at_i.ins, last_sd2.ins, sync=False,
                               reason="trig block after absrsqrt block")
                rg = sqpool.tile([128, CF], f32, tag="sq", name=f"rg{c}")
                nc.vector._custom_dve(
                    COPS['RGSIN'], out=rg[:], in0=w[:],
                    in1=r_chunks[c][:],
                    s0=-1.0 / 3.0, s1=1.0 / 162.0, imm2=-1.0 / 29160.0)
                ot = oopool.tile([128, CF], odt, tag="oo", name=f"o{c}")
                nc.gpsimd.tensor_tensor(ot[:], rg[:], s_chunks[c][:],
                                        Alu.add)
                dview = OUT[512 * c:512 * (c + 1), :].rearrange(
                    "(tt p) n -> p tt n", p=128)
                nc.sync.dma_start(
                    dview, ot[:].rearrange("p (tt n) -> p tt n", n=256))

    nc.compile()
    return nc


def _get_program():
    key = (B_TOTAL, N, N_CORES)
    if key not in _PROG:
        _PROG[key] = _build_program()
    return _PROG[key]


def _host_prep(inputs):
    import ml_dtypes
    bf = ml_dtypes.bfloat16
    x = _np_f32(inputs['x']).reshape(B_TOTAL, N)
    x_b = _np_f32(inputs['x_b']).reshape(B_TOTAL, N)
    m = float(np.asarray(inputs['mass']).reshape(-1)[0])
    gp = float(np.asarray(inputs['gamma_p']).reshape(-1)[0])
    gamma = float(np.log1p(np.exp(gp))) if gp < 30 else gp
    TtT = _np_f32(inputs['TtT'])
    DtD = _np_f32(inputs['DtD'])

    W_A = ((np.eye(N, dtype=np.float32) - np.float32(gamma) * TtT.T)
           / np.float32(3.0 * m)).astype(np.float32)
    W_B = (-np.float32(gamma) * DtD.T / np.float32(3.0 * m)).astype(np.float32)
    WM = np.concatenate([W_A, W_B], axis=1).astype(bf)          # (256,512)

    M1s, M2s, lws = {}, {}, {}
    for tag in ('mu', 'reg'):
        M1s[tag] = _conv_pool_mat(inputs['w2_' + tag], 256)      # (64,256)
        M2s[tag] = _conv_pool_mat(inputs['w3_' + tag], 64)       # (16,64)
        lws[tag] = _np_f32(inputs['lw_' + tag]).reshape(16)
    M1cat = np.concatenate([M1s['mu'], M1s['reg']], axis=0)      # (128,256)
    M1T = np.ascontiguousarray(M1cat.T).astype(bf)               # (256,128)
    M2BD = np.zeros((128, 32), np.float32)
    M2BD[0:64, 0:16] = M2s['mu'].T
    M2BD[64:128, 16:32] = M2s['reg'].T
    M2BD = M2BD.astype(bf)
    LWBD1 = np.zeros((32, 2), np.float32)
    LWBD1[0:16, 0] = lws['mu']
    LWBD1[16:32, 1] = lws['reg']
    LWBD = np.tile(LWBD1, (4, 1)).astype(bf)                     # (128,2)

    def sc(name):
        return float(np.asarray(inputs[name]).reshape(-1)[0])

    B2V = np.full((128, 1), sc('b2_mu'), np.float32)
    B2V[64:] = sc('b2_reg')
    B3V1 = np.full((32, 1), sc('b3_mu'), np.float32)
    B3V1[16:] = sc('b3_reg')
    B3V = np.tile(B3V1, (4, 1))                                  # (128,1)
    LBM = np.full((128, 1), sc('lb_mu'), np.float32)
    LBR = np.full((128, 1), sc('lb_reg'), np.float32)
    GSC = np.full((128, 1), gamma / (m * m), np.float32)

    EYEM = np.eye(128, dtype=np.float32).astype(bf)
    consts = dict(wm=WM, m1t=M1T, m2bd=M2BD, lwbd=LWBD, eye=EYEM,
                  b2v=B2V, b3v=B3V, lbm=LBM, lbr=LBR, gsc=GSC)

    xb3 = (np.float32(gamma / (3.0 * m)) * x_b).astype(bf)   # centered
    xbf = x.astype(bf)
    in_maps = []
    for c in range(N_CORES):
        rows = slice(BC * c, BC * (c + 1))
        im = dict(consts)
        im['xt'] = np.ascontiguousarray(xbf[rows].T)
        im['xbt'] = np.ascontiguousarray(xb3[rows].T)
        in_maps.append(im)
    return in_maps, m


def kernel(**inputs) -> np.ndarray:
    from concourse import bass_utils
    nc = _get_program()
    in_maps, m = _host_prep(inputs)
    res = bass_utils.run_bass_kernel_spmd(nc, in_maps,
                                          core_ids=list(range(N_CORES)))
    out = np.concatenate([res.results[c]['out'] for c in range(N_CORES)],
                         axis=0).astype(np.float32)
    if m != 1.0:
        out = (np.float32(m) * out).astype(np.float32)
    return np.ascontiguousarray(out.reshape(B_TOTAL, 1, N))
